# revision 20
# baseline (speedup 1.0000x reference)
"""L1 pairwise distance kernel for Trainium2, 8 NeuronCores.

res[i, j] = sum_d |x1c[i, d] - x2c[j, d]|,  x1c/x2c centered by mean(x1).

Per core: 256 x1 rows (data-parallel over rows), 2 halves of 128 rows =
64 row-pairs each, split across three producer paths sized so DVE / ACT
/ PE finish together. fp16/relu paths use |d| = 2 relu(d) - d, whose
linear term sum_d d = S2[j] - S1[i] is rank-1; the 2x rides in the mask
values and the rank-1 lands as one extra matmul per PSUM region.

  A-path (DVE, NA pairs/half): one fused tensor_scalar (add bias, max
    0) per pair -> R = relu(x2c^T - x1) fp16 at 4x (~0.74us); one-hot
    2.0-masks reduce into PSUM_A. fp16-exact.
  K-path (DVE, NK pairs/half): same relu but emitted in fp8e4 (1x mode,
    ~2.3us); pairs of K-tiles feed DoubleRow fp8 matmuls (256-deep
    contraction -> half the PE time per pair) into PSUM_J with 2.0
    masks. Rank-1 correction for K rows rides an extra DoubleRow group
    whose S1/S2 lines are split hi+lo across two fp8 partitions each
    (keeps the correction exact to ~0.08).
  J-path (ACT, NJ pairs/half): activation Abs emits |d| tiles directly
    in fp8e4; tile pairs feed DoubleRow fp8 matmuls into PSUM_J.

  fp8 rows carry ~1.4e-2 rel err (< 2e-2 gate). PSUM tiles are split
  into 4 column chunks so copies and output DMAs drain each chunk as
  its accumulation completes. PSUM_A copies on DVE, PSUM_J on ACT.

Self-contained: hardcodes shapes from the problem spec.
"""

import numpy as np
import ml_dtypes

import bass_rust
import concourse.bass as bass
import concourse.tile as tile
from concourse import mybir
import concourse.bass_utils as bu

N1 = 2048
N2 = 2048
D = 64
NCORES = 8
IPC = N1 // NCORES          # 256 x1 rows per core
NPAIR_HALF = 64
JCH = 512                   # matmul free-dim chunk (one PSUM bank)
NCH = N2 // JCH             # 4 column chunks
NA = 36                     # DVE fp16 relu pairs per half
NPRE = 3                    # DVE presum groups per half (4 rows each)
NK = 0                      # fp8 relu path disabled (2x fp8 err)
NJ = NPAIR_HALF - NA - NK - 2 * NPRE  # ACT fp8 pairs per half (even)
NGK = NK // 2
NGJ = NJ // 2
NAROWS = 2 * NA + 4 * NPRE  # rows in PSUM_A per half
F32 = mybir.dt.float32
F16 = mybir.dt.float16
F8 = mybir.dt.float8e4
A = mybir.AluOpType
DR = mybir.MatmulPerfMode.DoubleRow
ABS = mybir.ActivationFunctionType.Abs

_nop_counter = [0]


def _split_multi_waits(nc):
    """This container's walrus build allows one sync-wait per instruction.
    Move extra waits onto same-engine NoOps placed just before."""
    for fn in nc.m.functions:
        for blk in fn.blocks:
            out = []
            changed = False
            for inst in blk.instructions:
                si = inst.sync_info
                if si is not None and len(si.on_wait) > 1:
                    waits = list(si.on_wait)
                    for w in waits[:-1]:
                        _nop_counter[0] += 1
                        nop = mybir.InstNoOp(
                            name=f"I-waitsplit-{_nop_counter[0]}", ins=[], outs=[]
                        )
                        nop.engine = inst.engine
                        nop.sync_info = bass_rust.SyncInfo(on_wait=[w], on_update=[])
                        if inst.debug is not None:
                            nop.debug = inst.debug
                        out.append(nop)
                        nc.register_instruction(nop, overwrite=True)
                    si.on_wait = waits[-1:]
                    changed = True
                out.append(inst)
            if changed:
                blk.instructions = out


def _schedule(counts):
    """Proportional merge: yields (kind, idx) interleaved by fraction."""
    items = []
    for kind, n in counts.items():
        w = 0.8 if kind == "j" else 1.0
        for i in range(n):
            items.append(((i + 0.5) / n * w, kind, i))
    items.sort()
    return [(k, i) for _, k, i in items]


def _build():
    nc = bass.Bass()
    x2s16_d = nc.dram_tensor("x2s16", [128, N2], F16, kind="ExternalInput")
    x2e4_d = nc.dram_tensor("x2e4", [128, N2], F16, kind="ExternalInput")
    x2o4_d = nc.dram_tensor("x2o4", [128, N2], F16, kind="ExternalInput")
    x2sum4_d = nc.dram_tensor("x2sum4", [128, N2], F16, kind="ExternalInput")
    biase_d = nc.dram_tensor("biase", [128, 2 * NPRE], F32, kind="ExternalInput")
    biaso_d = nc.dram_tensor("biaso", [128, 2 * NPRE], F32, kind="ExternalInput")
    biass_d = nc.dram_tensor("biass", [128, 2 * NPRE], F32, kind="ExternalInput")
    maskp16_d = nc.dram_tensor("maskp16", [128, NPRE, 128], F16, kind="ExternalInput")
    bias_d = nc.dram_tensor("bias", [128, IPC // 2], F32, kind="ExternalInput")
    maskb16_d = nc.dram_tensor("maskb16", [128, 254], F16, kind="ExternalInput")
    kmask8_d = (nc.dram_tensor("kmask8", [128, NGK, 2, 128], F8, kind="ExternalInput")
                if NGK else None)
    jmask8_d = nc.dram_tensor("jmask8", [128, NGJ, 2, 128], F8, kind="ExternalInput")
    kcorl_d = (nc.dram_tensor("kcorl", [128, 2, 2, 128], F8, kind="ExternalInput")
               if NGK else None)
    kcorr_d = (nc.dram_tensor("kcorr", [128, 2, N2], F8, kind="ExternalInput")
               if NGK else None)
    corrl_d = nc.dram_tensor("corrl", [128, 2, 128], F16, kind="ExternalInput")
    corrr_d = nc.dram_tensor("corrr", [128, N2], F16, kind="ExternalInput")
    out_d = nc.dram_tensor("out", [IPC, N2], F32, kind="ExternalOutput")

    with tile.TileContext(nc) as tc:
        with (
            tc.tile_pool(name="singles", bufs=1) as singles,
            tc.tile_pool(name="ad", bufs=8) as adpool,
            tc.tile_pool(name="kd", bufs=2) as kdpool,
            tc.tile_pool(name="pre", bufs=2) as prepool,
            tc.tile_pool(name="jd", bufs=3) as jdpool,
            tc.tile_pool(name="psa", bufs=1, space="PSUM") as psapool,
            tc.tile_pool(name="psj", bufs=1, space="PSUM") as psjpool,
            tc.tile_pool(name="ob", bufs=2) as outpool,
        ):
            def load_split(dram, tile_, nsplit):
                w = N2 // nsplit
                for q in range(nsplit):
                    nc.sync.dma_start(
                        tile_[:, q * w : (q + 1) * w], dram[:, q * w : (q + 1) * w]
                    )

            x2s16 = singles.tile([128, N2], F16)
            load_split(x2s16_d, x2s16, 8)
            x2e4 = singles.tile([128, N2], F16)
            load_split(x2e4_d, x2e4, 2)
            x2o4 = singles.tile([128, N2], F16)
            load_split(x2o4_d, x2o4, 2)
            x2sum4 = singles.tile([128, N2], F16)
            load_split(x2sum4_d, x2sum4, 2)
            biase = singles.tile([128, 2 * NPRE], F32)
            nc.sync.dma_start(biase[:], biase_d[:])
            biaso = singles.tile([128, 2 * NPRE], F32)
            nc.sync.dma_start(biaso[:], biaso_d[:])
            biass = singles.tile([128, 2 * NPRE], F32)
            nc.sync.dma_start(biass[:], biass_d[:])
            maskp16 = singles.tile([128, NPRE, 128], F16)
            nc.sync.dma_start(maskp16[:], maskp16_d[:])
            bias = singles.tile([128, IPC // 2], F32)
            nc.sync.dma_start(bias[:], bias_d[:])
            maskb16 = singles.tile([128, 254], F16)
            nc.sync.dma_start(maskb16[:], maskb16_d[:])
            if NGK:
                kmask8 = singles.tile([128, NGK, 2, 128], F8)
                nc.sync.dma_start(kmask8[:], kmask8_d[:])
            jmask8 = singles.tile([128, NGJ, 2, 128], F8)
            nc.sync.dma_start(jmask8[:], jmask8_d[:])
            if NGK:
                kcorl = singles.tile([128, 2, 2, 128], F8)
                nc.sync.dma_start(kcorl[:], kcorl_d[:])
                kcorr = singles.tile([128, 2, N2], F8)
                nc.sync.dma_start(kcorr[:], kcorr_d[:])
            corrl = singles.tile([128, 2, 128], F16)
            nc.sync.dma_start(corrl[:], corrl_d[:])
            corrr = singles.tile([128, N2], F16)
            nc.sync.dma_start(corrr[:], corrr_d[:])

            for h in range(2):
                psA = [psapool.tile([128, JCH], F32, tag=f"psA{jc}",
                                    name=f"psA{jc}_{h}")
                       for jc in range(NCH)]
                psJ = [psjpool.tile([128, JCH], F32, tag=f"psJ{jc}",
                                    name=f"psJ{jc}_{h}")
                       for jc in range(NCH)]
                fp8_done = 0
                psa_started = False
                nfp8 = NGK + NGJ
                for kind, idx in _schedule({"a": NA, "p": NPRE, "j": NGJ}):
                    if kind == "a":
                        a = idx
                        pi = h * NPAIR_HALF + a
                        ad = adpool.tile([128, N2], F16, tag="ad")
                        nc.vector.tensor_scalar(
                            out=ad[:], in0=x2s16[:],
                            scalar1=bias[:, pi : pi + 1], scalar2=0.0,
                            op0=A.add, op1=A.max,
                        )
                        mg = maskb16[:, 126 - 2 * a : 254 - 2 * a]
                        for jc in range(NCH):
                            nc.tensor.matmul(
                                psA[jc][:],
                                mg,
                                ad[:, jc * JCH : (jc + 1) * JCH],
                                start=not psa_started,
                                stop=False,
                            )
                        psa_started = True
                    elif kind == "p":
                        g = idx
                        col = h * NPRE + g
                        rE = prepool.tile([128, N2], F16, tag="rE")
                        nc.vector.tensor_scalar(
                            out=rE[:], in0=x2e4[:],
                            scalar1=biase[:, col : col + 1], scalar2=0.0,
                            op0=A.add, op1=A.max,
                        )
                        q2 = prepool.tile([128, N2], F16, tag="q2")
                        nc.vector.scalar_tensor_tensor(
                            out=q2[:], in0=x2o4[:],
                            scalar=biaso[:, col : col + 1], in1=rE[:],
                            op0=A.add, op1=A.max,
                        )
                        mp = prepool.tile([128, N2], F16, tag="mp")
                        nc.vector.scalar_tensor_tensor(
                            out=mp[:], in0=x2sum4[:],
                            scalar=biass[:, col : col + 1], in1=q2[:],
                            op0=A.add, op1=A.max,
                        )
                        for jc in range(NCH):
                            nc.tensor.matmul(
                                psA[jc][:],
                                maskp16[:, g, :],
                                mp[:, jc * JCH : (jc + 1) * JCH],
                                start=not psa_started,
                                stop=False,
                            )
                        psa_started = True
                    elif kind == "k":
                        g = idx
                        kt = kdpool.tile([128, 2, N2], F8, tag="kt")
                        for i in range(2):
                            pi = h * NPAIR_HALF + NA + 2 * g + i
                            nc.vector.tensor_scalar(
                                out=kt[:, i, :], in0=x2s16[:],
                                scalar1=bias[:, pi : pi + 1], scalar2=0.0,
                                op0=A.add, op1=A.max,
                            )
                        for jc in range(NCH):
                            nc.tensor.matmul(
                                psJ[jc][:],
                                kmask8[:, g, :, :],
                                kt[:, :, jc * JCH : (jc + 1) * JCH],
                                start=(fp8_done == 0),
                                stop=False,
                                perf_mode=DR,
                            )
                        fp8_done += 1
                    else:
                        g = idx
                        jt = jdpool.tile([128, 2, N2], F8, tag="jt")
                        for i in range(2):
                            pi = h * NPAIR_HALF + NA + 2 * NPRE + NK + 2 * g + i
                            nc.scalar.activation(
                                out=jt[:, i, :], in_=x2s16[:],
                                func=ABS,
                                bias=bias[:, pi : pi + 1], scale=1.0,
                            )
                        for jc in range(NCH):
                            nc.tensor.matmul(
                                psJ[jc][:],
                                jmask8[:, g, :, :],
                                jt[:, :, jc * JCH : (jc + 1) * JCH],
                                start=(fp8_done == 0),
                                stop=(NGK == 0 and fp8_done == nfp8 - 1),
                                perf_mode=DR,
                            )
                        fp8_done += 1

                # rank-1 corrections + chunk drains (chunk order jc)
                for jc in range(NCH):
                    # psA: += S1[m] - S2[j] over fp16 lines
                    nc.tensor.matmul(
                        psA[jc][:],
                        corrl[:, h, :],
                        corrr[:, jc * JCH : (jc + 1) * JCH],
                        start=False,
                        stop=True,
                    )
                    if NGK:
                        # psJ: K rows correction, S1/S2 as fp8 hi+lo pairs
                        nc.tensor.matmul(
                            psJ[jc][:],
                            kcorl[:, h, :, :],
                            kcorr[:, :, jc * JCH : (jc + 1) * JCH],
                            start=False,
                            stop=True,
                            perf_mode=DR,
                        )
                    obA = outpool.tile([128, JCH], F32, tag=f"obA{jc}")
                    obJ = outpool.tile([128, JCH], F32, tag=f"obJ{jc}")
                    nc.vector.tensor_copy(obA[:], psA[jc][:])
                    nc.scalar.copy(obJ[:], psJ[jc][:])
                    nc.sync.dma_start(
                        out_d[h * 128 : h * 128 + NAROWS,
                              jc * JCH : (jc + 1) * JCH],
                        obA[0:NAROWS, :],
                    )
                    nc.sync.dma_start(
                        out_d[h * 128 + NAROWS : (h + 1) * 128,
                              jc * JCH : (jc + 1) * JCH],
                        obJ[NAROWS:128, :],
                    )
    _split_multi_waits(nc)
    return nc


_cached_nc = None


def _hilo8(v):
    hi = v.astype(ml_dtypes.float8_e4m3fn).astype(np.float32)
    lo = (v - hi).astype(ml_dtypes.float8_e4m3fn)
    return hi.astype(ml_dtypes.float8_e4m3fn), lo


def _prep_inputs(x1, x2):
    x1 = np.asarray(x1, dtype=np.float32)
    x2 = np.asarray(x2, dtype=np.float32)
    adj = x1.mean(axis=0, dtype=np.float32).astype(np.float32)
    x1c = x1 - adj
    x2c = x2 - adj

    x2s16 = np.tile(np.ascontiguousarray(x2c.T), (2, 1)).astype(np.float16)
    x2e = np.ascontiguousarray(x2c.T[0:32])       # [32, N2] dims 0..31
    x2o = np.ascontiguousarray(x2c.T[32:64])      # dims 32..63
    x2e4 = np.tile(x2e, (4, 1)).astype(np.float16)
    x2o4 = np.tile(x2o, (4, 1)).astype(np.float16)
    x2sum4 = np.tile(x2e + x2o, (4, 1)).astype(np.float16)

    k = np.arange(128)
    maskb = np.zeros((128, 254), dtype=np.float32)
    maskb[k, 126 + k // 64] = 2.0
    maskb16 = maskb.astype(np.float16)

    # presum masks: group g rows m0..m0+3, m0 = 2NA + 4g; 2.0 at m0 + k//32
    maskp16 = np.zeros((128, NPRE, 128), dtype=np.float16)
    for g in range(NPRE):
        maskp16[k, g, 2 * NA + 4 * g + k // 32] = 2.0
    # J: group g covers rows NAROWS + 4g .. +3
    jmask8 = np.zeros((128, NGJ, 2, 128), dtype=ml_dtypes.float8_e4m3fn)
    for g in range(NGJ):
        for i in range(2):
            jmask8[k, g, i, NAROWS + 4 * g + 2 * i + k // 64] = 1.0

    S2 = x2c.sum(axis=1, dtype=np.float32)            # [N2]
    corrr = np.zeros((128, N2), dtype=np.float16)
    corrr[0, :] = S2.astype(np.float16)
    corrr[1, :] = 1.0
    # fp8 correction rhs (DoubleRow): ktile0 p0/p1 = S2 hi/lo, p2 = 1
    s2hi, s2lo = _hilo8(S2)
    kcorr = np.zeros((128, 2, N2), dtype=ml_dtypes.float8_e4m3fn)
    kcorr[0, 0, :] = s2hi
    kcorr[1, 0, :] = s2lo
    kcorr[2, 0, :] = 1.0
    kcorr[3, 0, :] = 1.0

    in_maps = []
    for c in range(NCORES):
        sl = x1c[c * IPC : (c + 1) * IPC]          # [256, 64]
        b = -np.transpose(sl.reshape(IPC // 2, 2, D), (1, 2, 0)).reshape(128, IPC // 2)
        S1 = sl.sum(axis=1, dtype=np.float32)      # [256]
        corrl = np.zeros((128, 2, 128), dtype=np.float16)
        for h in range(2):
            corrl[0, h, 0:NAROWS] = -1.0
            corrl[1, h, 0:NAROWS] = S1[h * 128 : h * 128 + NAROWS].astype(np.float16)
        biase = np.zeros((128, 2 * NPRE), dtype=np.float32)
        biaso = np.zeros((128, 2 * NPRE), dtype=np.float32)
        biass = np.zeros((128, 2 * NPRE), dtype=np.float32)
        cc = k // 32
        dp = k % 32
        for h in range(2):
            for g in range(NPRE):
                i0 = h * 128 + 2 * NA + 4 * g
                col = h * NPRE + g
                biase[:, col] = -sl[i0 + cc, dp]
                biaso[:, col] = -sl[i0 + cc, 32 + dp]
                biass[:, col] = biase[:, col] + biaso[:, col]
        in_maps.append({
            "x2s16": x2s16,
            "x2e4": x2e4,
            "x2o4": x2o4,
            "x2sum4": x2sum4,
            "bias": np.ascontiguousarray(b, dtype=np.float32),
            "biase": biase,
            "biaso": biaso,
            "biass": biass,
            "maskb16": maskb16,
            "maskp16": maskp16,
            "jmask8": jmask8.view(np.uint8),
            "corrl": corrl,
            "corrr": corrr,
        })
    return in_maps


def run(x1, x2, trace=False):
    global _cached_nc
    if _cached_nc is None:
        _cached_nc = _build()
    in_maps = _prep_inputs(x1, x2)
    r = bu.run_bass_kernel_spmd(
        _cached_nc, in_maps, core_ids=list(range(NCORES)), trace=trace
    )
    out = np.concatenate([r.results[c]["out"] for c in range(NCORES)], axis=0)
    return out, r


def kernel(x1, x2):
    out, _ = run(x1, x2, trace=False)
    return out
